# revision 8
# baseline (speedup 1.0000x reference)
"""BiMatchLoss kernel for Trainium2 (8 NeuronCores, SPMD data-parallel over batch).

Math (validated vs reference):
  BCE(p,t) = -log1mp(p) - t*(logp(p) - log1mp(p))
  Summed over a bijective matching perm, the -log1mp part is perm-independent.
  Per batch b the device computes (one pass over the data):
    cost[t,o]  = -sum_{s,ci} tgt[s,t,ci] * out[s,o,ci]          (argmin input)
    G[t,o]     =  sum_{s,ci} tgt[s,t,ci] * m[s]*D[s,o,ci]       (D = logit)
    Amask      =  sum_{s,o,ci} m[s] * (-log1mp[s,o,ci])
  final = sum_b 0.5*(Amask_b - sum_t G[t, perm_b[t]]) / sum(m)

Device layout per batch (S=1024 -> 8 s-tiles of 128 partitions, F=192):
  comb [128, 8*384] bf16: per-tile blocks [out_k | m*D_k]  (matmul rhs)
  xt_f [128, 8*192] bf16: targets                          (matmul lhsT)
  2 matmuls per s-tile (M-split 0:128 / 64:192), N=384, PSUM-accumulated
  over the 8 tiles into a 2-bank PSUM tile -> block-diag mask + grouped
  reduce -> [128, 24] partials. Host does the 720-permutation argmin and
  final scalar assembly.
"""

import os
from itertools import permutations

import numpy as np
import ml_dtypes

import concourse.bacc as bacc
import concourse.mybir as mybir
from concourse.tile import TileContext
from concourse.bass_utils import run_bass_kernel_spmd

B, S, E, C = 32, 1024, 6, 16
F = E * C * 2          # 192 flattened (e, c, i)
CI = C * 2             # 32
NCORE = 8
NB = B // NCORE        # 4 batches per core
NT = S // 128          # 8 s-tiles per batch

f32 = mybir.dt.float32
bf16 = mybir.dt.bfloat16
AF = mybir.ActivationFunctionType
ALU = mybir.AluOpType
AX = mybir.AxisListType

_PROG = None           # cached compiled Bass program
LAST = None            # last BassKernelResults (for test.py timing)


def _build_program():
    nc = bacc.Bacc("TRN2", target_bir_lowering=False, debug=False,
                   num_devices=1)

    xo_d = nc.dram_tensor("xo", [NB, S, F], bf16, kind="ExternalInput").ap()
    xt_d = nc.dram_tensor("xt", [NB, S, F], bf16, kind="ExternalInput").ap()
    mcol_d = nc.dram_tensor("mcol", [128, NB * NT], f32,
                            kind="ExternalInput").ap()
    dmask_d = nc.dram_tensor("dmask", [128, 768], bf16,
                             kind="ExternalInput").ap()
    red_d = nc.dram_tensor("red", [NB, 128, 24], f32,
                           kind="ExternalOutput").ap()
    amask_d = nc.dram_tensor("amask", [NB, 128], f32,
                             kind="ExternalOutput").ap()

    with TileContext(nc) as tc:
        with (
            tc.tile_pool(name="consts", bufs=1) as cpool,
            tc.tile_pool(name="io", bufs=2) as iop,
            tc.tile_pool(name="mid", bufs=2) as midp,
            tc.tile_pool(name="post", bufs=2) as postp,
            tc.tile_pool(name="ps", bufs=2, space="PSUM") as psp,
        ):
            mcol_sb = cpool.tile([128, NB * NT], f32)
            nc.sync.dma_start(mcol_sb[:], mcol_d)
            dmask_sb = cpool.tile([128, 768], bf16)
            nc.sync.dma_start(dmask_sb[:], dmask_d)

            for b in range(NB):
                # comb: per-tile [out_k (192) | m*D_k (192)] blocks
                comb = iop.tile([128, NT * 384], bf16, tag="comb")
                comb_v = comb[:].rearrange("p (k q) -> p k q", q=384)
                # xo -> strided blocks comb[:, k*384 : k*384+192]
                xo_b = xo_d[b].rearrange("(k p) f -> p k f", p=128)
                for h in range(2):
                    nc.sync.dma_start(comb_v[:, h * 4:(h + 1) * 4, 0:F],
                                      xo_b[:, h * 4:(h + 1) * 4, :])
                xt_f = iop.tile([128, NT * F], bf16, tag="xt_f")
                xt_fv = xt_f[:].rearrange("p (k f) -> p k f", f=F)
                xt_b = xt_d[b].rearrange("(k p) f -> p k f", p=128)
                for h in range(2):
                    nc.sync.dma_start(xt_fv[:, h * 4:(h + 1) * 4, :],
                                      xt_b[:, h * 4:(h + 1) * 4, :])

                # logs: cols 0:1536 = log(p), 1536:3072 = log(1-p)
                logs = midp.tile([128, 2 * NT * F], bf16, tag="logs")
                nc.scalar.activation(logs[:, 0:1536], comb_v[:, :, 0:F], AF.Ln)
                nc.scalar.activation(logs[:, 1536:3072], comb_v[:, :, 0:F],
                                     AF.Ln, bias=1.0, scale=-1.0)

                # D = logp - log1mp -> comb[:, k*384+192 : (k+1)*384]
                nc.vector.tensor_sub(comb_v[:, :, F:384],
                                     logs[:, 0:1536], logs[:, 1536:3072])
                # mask D in place: per-tile per-partition scalar m
                for k in range(NT):
                    blk = comb[:, k * 384 + F:(k + 1) * 384]
                    nc.gpsimd.tensor_scalar(
                        blk, blk, mcol_sb[:, b * NT + k:b * NT + k + 1],
                        None, ALU.mult)

                # matmuls: 2 per s-tile, accumulate over tiles
                ps = psp.tile([128, 1024], f32, tag="ps")
                for k in range(NT):
                    st = dict(start=(k == 0), stop=(k == NT - 1))
                    rhs = comb[:, k * 384:(k + 1) * 384]
                    nc.tensor.matmul(ps[:, 0:384],
                                     xt_f[:, k * F:k * F + 128], rhs, **st)
                    nc.tensor.matmul(ps[:, 512:896],
                                     xt_f[:, k * F + 64:(k + 1) * F], rhs, **st)

                # block-diag extraction
                ps_v = ps[:].rearrange("p (h q) -> p h q", q=512)[:, :, 0:384]
                tmp = postp.tile([128, 768], bf16, tag="tmp")
                nc.vector.tensor_tensor(tmp[:], ps_v, dmask_sb[:], ALU.mult)
                red_sb = postp.tile([128, 24], f32, tag="red_sb")
                nc.vector.tensor_reduce(
                    red_sb[:], tmp[:].rearrange("p (g j) -> p g j", j=CI),
                    AX.X, ALU.add)
                nc.sync.dma_start(red_d[b], red_sb[:])

                # Amask partials: arow[p,k] = sum_f log1mp; dot with m cols
                arow = postp.tile([128, NT], f32, tag="arow")
                nc.vector.tensor_reduce(
                    arow[:], logs[:, 1536:3072].rearrange(
                        "p (k f) -> p k f", f=F),
                    AX.X, ALU.add)
                junk = postp.tile([128, NT], f32, tag="junk")
                nc.vector.tensor_tensor(junk[:], arow[:],
                                        mcol_sb[:, b * NT:(b + 1) * NT],
                                        ALU.mult)
                am_col = postp.tile([128, 1], f32, tag="am_col")
                nc.vector.tensor_reduce(am_col[:], junk[:], AX.X, ALU.add)
                nc.sync.dma_start(amask_d[b, :], am_col[:])

    nc.compile()
    return nc


def _get_program():
    global _PROG
    if _PROG is None:
        _PROG = _build_program()
    return _PROG


def kernel(outputs, targets, attention_mask):
    global LAST
    out_np = np.asarray(outputs, dtype=np.float32)
    tgt_np = np.asarray(targets, dtype=np.float32)
    m_np = np.asarray(attention_mask)

    xo_all = out_np.reshape(B, S, F).astype(ml_dtypes.bfloat16)
    xt_all = tgt_np.reshape(B, S, F).astype(ml_dtypes.bfloat16)

    # dmask[p, q] = 1 where p%32 == q%32 (block-diagonal selector)
    p_idx = np.arange(128)[:, None] % CI
    q_idx = np.arange(768)[None, :] % CI
    dmask = (p_idx == q_idx).astype(ml_dtypes.bfloat16)

    in_maps = []
    for c in range(NCORE):
        bs = slice(c * NB, (c + 1) * NB)
        m_core = m_np[bs].astype(np.float32)          # [NB, S]
        # mcol[p, b*NT+k] = m[b, k*128+p]
        mcol = np.ascontiguousarray(
            m_core.reshape(NB, NT, 128).transpose(2, 0, 1).reshape(128, NB * NT))
        in_maps.append({
            "xo": np.ascontiguousarray(xo_all[bs]),
            "xt": np.ascontiguousarray(xt_all[bs]),
            "mcol": mcol,
            "dmask": dmask,
        })

    nc = _get_program()
    res = run_bass_kernel_spmd(nc, in_maps, list(range(NCORE)))
    LAST = res

    P = np.array(list(permutations(range(E))), dtype=np.int32)
    t_idx = np.arange(E)[None, :]
    ar = np.arange(E)
    num = 0.0
    for c in range(NCORE):
        red = res.results[c]["red"]      # [NB, 128, 24] f32
        am = res.results[c]["amask"]     # [NB, 128] f32
        for b in range(NB):
            rb = red[b]
            # groups: 0:6 cost-hi, 6:12 G-hi (rows t0..3 x j);
            #         12:18 cost-lo, 18:24 G-lo (rows 64:128 = t4,5 x j)
            hi = rb[:, 0:12].reshape(4, 32, 12).sum(1, dtype=np.float32)
            lo = rb[64:128, 12:24].reshape(2, 32, 12).sum(1, dtype=np.float32)
            cost = -np.concatenate([hi[:, 0:6], lo[:, 0:6]], axis=0)
            G = np.concatenate([hi[:, 6:12], lo[:, 6:12]], axis=0)

            totals = cost[t_idx, P].sum(-1, dtype=np.float32)
            perm = P[int(np.argmin(totals))]
            amask_b = -am[b].sum(dtype=np.float64)
            num += 0.5 * (amask_b - float(G[ar, perm].sum(dtype=np.float64)))

    den = float(m_np.sum())
    return np.float32(num / den)


# revision 18
# speedup vs baseline: 1.9838x; 1.9838x over previous
"""BiMatchLoss kernel for Trainium2 (8 NeuronCores, SPMD data-parallel over batch).

Math (validated vs reference):
  BCE(p,t) = -log1mp(p) - t*(logp(p) - log1mp(p))
  Summed over a bijective matching perm, the -log1mp part is perm-independent.
  Per batch b the device computes (one pass over the data):
    cost[t,o]  = -sum_{s,ci} tgt[s,t,ci] * out[s,o,ci]            (argmin input)
    G[t,o]     =  sum_{s,ci} (m[s]*tgt[s,t,ci]) * D'[s,o,ci]
    Amask      =  sum_{s,o,ci} m[s] * (-log1mp[s,o,ci])
  where D' = logp - m*log1mp equals the logit wherever the mask is 1; masked
  rows are zeroed by the host-premasked targets (m*tgt). The mask products
  (m*tgt, m*out) are exact host-side preprocessing (bf16-exact binaries).
  final = sum_b 0.5*(Amask_b - sum_t G[t, perm_b[t]]) / sum(m)

Device per batch: 2 fused Ln ops (ACT; the log(1-x) op also yields the Amask
row-sums via accum_out), one fused D' subtract (DVE), 32 bf16 matmuls (K=128
per s-tile, PSUM-accumulated over 8 tiles, one accumulation group per PSUM
bank), block-diag mask + grouped reduce -> [128,24] partials. Batches are
software-pipelined (prep of b+1 issued before matmuls of b). Host does the
720-permutation argmin and final scalar assembly.
"""

import os
from itertools import permutations

import numpy as np
import ml_dtypes

import concourse.bacc as bacc
import concourse.mybir as mybir
from concourse.tile import TileContext
from concourse.bass_utils import run_bass_kernel_spmd

B, S, E, C = 32, 1024, 6, 16
F = E * C * 2          # 192 flattened (e, c, i)
CI = C * 2             # 32
NCORE = 8
NB = B // NCORE        # 4 batches per core
NT = S // 128          # 8 s-tiles per batch

f32 = mybir.dt.float32
bf16 = mybir.dt.bfloat16
fp8 = mybir.dt.float8e4
AF = mybir.ActivationFunctionType
ALU = mybir.AluOpType
AX = mybir.AxisListType

_PROG = None           # cached compiled Bass program
LAST = None            # last BassKernelResults (for test.py timing)


def _build_program():
    nc = bacc.Bacc("TRN2", target_bir_lowering=False, debug=False,
                   num_devices=1)

    xo_d = nc.dram_tensor("xo", [NB, S, F], bf16, kind="ExternalInput").ap()
    xoz_d = nc.dram_tensor("xoz", [NB, S, F], bf16, kind="ExternalInput").ap()
    xt_d = nc.dram_tensor("xt", [NB, S, F], fp8, kind="ExternalInput").ap()
    xtm_d = nc.dram_tensor("xtm", [NB, S, F], fp8, kind="ExternalInput").ap()
    dmask_d = nc.dram_tensor("dmask", [128, 768], bf16,
                             kind="ExternalInput").ap()
    red_d = nc.dram_tensor("red", [NB, 128, 24], f32,
                           kind="ExternalOutput").ap()
    amask_d = nc.dram_tensor("amask", [NB, 128], f32,
                             kind="ExternalOutput").ap()

    with TileContext(nc) as tc:
        with (
            tc.tile_pool(name="consts", bufs=1) as cpool,
            tc.tile_pool(name="io", bufs=2) as iop,
            tc.tile_pool(name="mid", bufs=2) as midp,
            tc.tile_pool(name="post", bufs=2) as postp,
            tc.tile_pool(name="ps", bufs=2, space="PSUM") as psp,
        ):
            dmask_sb = cpool.tile([128, 768], bf16)
            nc.sync.dma_start(dmask_sb[:], dmask_d)

            def load_tiled(tag, src, dt, eng):
                """DRAM [S,F] -> SBUF [128, NT*F], col block k = s-tile k.
                One DMA per tensor (internally split across 16 SDMA slots);
                eng picks the HWDGE queue (sync vs scalar) for parallelism."""
                t = iop.tile([128, NT * F], dt, tag=tag, name=tag)
                tv = t[:].rearrange("p (k f) -> p k f", f=F)
                sv = src.rearrange("(k p) f -> p k f", p=128)
                eng.dma_start(tv[:], sv[:])
                return t

            def prep(b):
                """Loads + logs + D' + Amask accum for batch b."""
                xo_f = load_tiled("xo_f", xo_d[b], bf16, nc.sync)
                xoz_f = load_tiled("xoz_f", xoz_d[b], bf16, nc.scalar)
                xt_f = load_tiled("xt_f", xt_d[b], fp8, nc.sync)
                xtm_f = load_tiled("xtm_f", xtm_d[b], fp8, nc.scalar)

                # logs: cols 0:1536 = log(p); 1536:3072 = m*log(1-p)
                # (log1p(-m*p) = 0 where m=0); accum -> Amask partials
                logs = midp.tile([128, 2 * NT * F], bf16, tag="logs")
                am_col = postp.tile([128, 1], f32, tag="am_col")
                nc.scalar.activation(logs[:, 0:1536], xo_f[:], AF.Ln)
                nc.scalar.activation(logs[:, 1536:3072], xoz_f[:], AF.Ln,
                                     bias=1.0, scale=-1.0,
                                     accum_out=am_col[:])
                nc.sync.dma_start(amask_d[b, :], am_col[:])
                # D' = logp - m*log1mp (correct logit wherever m=1)
                d_f = midp.tile([128, NT * F], bf16, tag="d_f")
                nc.vector.tensor_sub(d_f[:], logs[:, 0:1536],
                                     logs[:, 1536:3072])
                return xo_f, xt_f, xtm_f, d_f

            def mms(b, xo_f, xt_f, xtm_f, d_f):
                # 4 matmuls per s-tile, accumulated over the 8 tiles; one
                # accumulation group per PSUM bank (cols h*512:h*512+192):
                # cost-hi (M=128), cost-lo (M=64), G-hi (M=128), G-lo (M=64)
                ps = psp.tile([128, 2048], f32, tag="ps")
                nc.vector.memset(ps[64:128, 512:704], 0.0)
                nc.vector.memset(ps[64:128, 1536:1728], 0.0)
                for k in range(NT):
                    st = dict(start=(k == 0), stop=(k == NT - 1))
                    xo_k = xo_f[:, k * F:(k + 1) * F]
                    nc.tensor.matmul(ps[:, 0:192],
                                     xt_f[:, k * F:k * F + 128], xo_k, **st)
                    nc.tensor.matmul(ps[0:64, 512:704],
                                     xt_f[:, k * F + 128:(k + 1) * F], xo_k,
                                     **st)
                for k in range(NT):
                    st = dict(start=(k == 0), stop=(k == NT - 1))
                    d_k = d_f[:, k * F:(k + 1) * F]
                    nc.tensor.matmul(ps[:, 1024:1216],
                                     xtm_f[:, k * F:k * F + 128], d_k, **st)
                    nc.tensor.matmul(ps[0:64, 1536:1728],
                                     xtm_f[:, k * F + 128:(k + 1) * F], d_k,
                                     **st)
                return ps

            def post(b, ps):
                # block-diag extraction -> [128, 24] partials
                ps_v = ps[:].rearrange("p (h q) -> p h q", q=512)[:, :, 0:192]
                tmp = postp.tile([128, 768], bf16, tag="tmp")
                nc.vector.tensor_tensor(tmp[:], ps_v, dmask_sb[:], ALU.mult)
                red_sb = postp.tile([128, 24], f32, tag="red_sb")
                nc.vector.tensor_reduce(
                    red_sb[:], tmp[:].rearrange("p (g j) -> p g j", j=CI),
                    AX.X, ALU.add)
                nc.sync.dma_start(red_d[b], red_sb[:])

            state = prep(0)
            pss = None
            for b in range(NB):
                nxt = prep(b + 1) if b + 1 < NB else None
                ps = mms(b, *state)
                post(b, ps)
                state = nxt

    nc.compile()
    return nc


def _get_program():
    global _PROG
    if _PROG is None:
        _PROG = _build_program()
    return _PROG


def kernel(outputs, targets, attention_mask):
    global LAST
    out_np = np.asarray(outputs, dtype=np.float32)
    tgt_np = np.asarray(targets, dtype=np.float32)
    m_np = np.asarray(attention_mask)

    mf = m_np.astype(np.float32)[:, :, None]
    xo_all = out_np.reshape(B, S, F).astype(ml_dtypes.bfloat16)
    # masked copies are exact in bf16 (x*1 or 0); binary targets are exact
    # even in fp8e4
    xoz_all = (out_np.reshape(B, S, F) * mf).astype(ml_dtypes.bfloat16)
    xt_all = tgt_np.reshape(B, S, F).astype(ml_dtypes.float8_e4m3fn)
    xtm_all = (tgt_np.reshape(B, S, F) * mf).astype(ml_dtypes.float8_e4m3fn)

    # dmask[p, q] = 1 where p%32 == q%32 (block-diagonal selector)
    p_idx = np.arange(128)[:, None] % CI
    q_idx = np.arange(768)[None, :] % CI
    dmask = (p_idx == q_idx).astype(ml_dtypes.bfloat16)

    in_maps = []
    for c in range(NCORE):
        bs = slice(c * NB, (c + 1) * NB)
        in_maps.append({
            "xo": np.ascontiguousarray(xo_all[bs]),
            "xoz": np.ascontiguousarray(xoz_all[bs]),
            "xt": np.ascontiguousarray(xt_all[bs]),
            "xtm": np.ascontiguousarray(xtm_all[bs]),
            "dmask": dmask,
        })

    nc = _get_program()
    res = run_bass_kernel_spmd(nc, in_maps, list(range(NCORE)))
    LAST = res

    P = np.array(list(permutations(range(E))), dtype=np.int32)
    t_idx = np.arange(E)[None, :]
    ar = np.arange(E)
    num = 0.0
    for c in range(NCORE):
        red = res.results[c]["red"]      # [NB, 128, 24] f32
        am = res.results[c]["amask"]     # [NB, 128] f32
        for b in range(NB):
            rb = red[b]
            # groups 0:6 cost-hi (rows t0..3 x j), 6:12 cost-lo (rows 0:64 =
            # t4,5 x j), 12:18 G-hi, 18:24 G-lo (rows 0:64)
            cost = -np.concatenate(
                [rb[:, 0:6].reshape(4, 32, 6).sum(1, dtype=np.float32),
                 rb[0:64, 6:12].reshape(2, 32, 6).sum(1, dtype=np.float32)],
                axis=0)
            G = np.concatenate(
                [rb[:, 12:18].reshape(4, 32, 6).sum(1, dtype=np.float32),
                 rb[0:64, 18:24].reshape(2, 32, 6).sum(1, dtype=np.float32)],
                axis=0)

            totals = cost[t_idx, P].sum(-1, dtype=np.float32)
            perm = P[int(np.argmin(totals))]
            amask_b = -am[b].sum(dtype=np.float64)
            num += 0.5 * (amask_b - float(G[ar, perm].sum(dtype=np.float64)))

    den = float(m_np.sum())
    return np.float32(num / den)
